# revision 21
# baseline (speedup 1.0000x reference)
"""CAPMemory loss kernel for 8 Trainium2 NeuronCores.

Sharding: camera-sharded -- core c owns memory[c], the batch is replicated
(the per-sample stats each core produces are tiny, so this moves 16x less
HBM traffic than batch-sharding the replicated 128 MiB memory bank).

Device, per core (fp8 e4m3 DoubleRow matmul, fp32 PSUM):
  S[b, l] = <x_norm[b], memory[c, l]> * FP8_SCALE^2      [1024, 2048]
  E       = exp(S / (FP8_SCALE^2 * T))  (ACT, bf16)
  zin[b]  = sum_l E[b, l]               (ACT accumulator, per 512-bank)
  cand    = top-8 of each 256-wide chunk of E -> 64 values/sample (DVE MAX8)

Schedule (derived from perfetto traces; 85us baseline -> 76us): the MM
stream runs at the 216ns N=512 DoubleRow issue roofline, so all wins
are at the edges. All input DMAs ride ONE HWDGE ring (Sync) in exact
consumption order -- HWDGE rings are FIFO, so each chunk's completion
sem fires at its byte-order position instead of near the end of the
whole 6.3MB transfer (two rings round-robin at packet granularity and
cost 2-3us of PE idle per fill chunk). Edge chunks are split (M0/M1/M2
halves, M7 quarters) to shave per-boundary sem lag. 11 garbage warm-up
matmuls keep the PE busy (HAM-warm) until the first chunk lands; batch
tiles 0/1 accumulate in chunk-arrival order across all eight PSUM
banks. PSUM is managed as eight independent one-bank pool tiles:
Tile's pool-reuse dependency is buffer-granular, so per-bank tiles let
a btile's bank-k matmuls start as soon as bank k of the btile two
steps back was exp-read -- not after its whole 4-bank exp/accum-read
chain (a ~5us PE stall + HAM re-throttle otherwise). Batch tiles 2..7
run their matmul groups nch-major so each bank's exp/max8/zin-accum
fires the moment its 8-chunk accumulation completes, leaving only one
bank's exp+max8+reduce+DMA after the kernel's final matmul.

Host merge:
  epos[c, b] = exp(<x8[b], m8[c, tgt_b]>/T') recomputed in f32 from the
  exact fp8 operands the device consumed; intra CE = log(zin) - log(epos)
  on the own-camera core. For the inter loss the positive's value is
  removed from its camera's candidate list (nearest match to epos), the
  8x64 candidates are merged, and the exact top-50 negatives feed the
  log-sum-exp. A global top-50 element can only be missing from the
  candidates if >=8 larger elements share its 256-chunk (P ~ 1e-5 per
  run, and the substitute is the next-ranked value, so the effect is
  ~1e-6 relative even then).
"""

import numpy as np

T = 0.05
HARD_NEG_K = 50
LOSS_WEIGHT = 0.5
N_CAMS = 8
L = 2048
D = 2048
B = 1024
NBT = 8          # batch tiles of 128
KC8 = 8          # contraction chunks of 256 (fp8 DoubleRow: 2 k-rows/cell)
FP8_SCALE = 32.0  # pre-scale before e4m3 cast (keeps values out of denormals)
NCH = 8          # candidate chunks per row
CHW = 256        # chunk width
NTOP = NCH * 8   # candidates shipped per camera (top-8 of each chunk)

_CACHE = {}


def _split_multi_waits(nc):
    """This container's walrus build rejects instructions carrying more than
    one sync wait ('Too many sync wait commands'). Hoist all but the last
    wait of each instruction onto same-engine Drain carriers placed just
    before it — semantically identical on an in-order engine stream."""
    import concourse.mybir as mybir

    n = 0
    for fn in nc.m.functions:
        for bb in fn.blocks:
            out = []
            for inst in bb.instructions:
                si = inst.sync_info
                if si is not None and si.on_wait and len(si.on_wait) > 1:
                    waits = list(si.on_wait)
                    for w in waits[:-1]:
                        d = mybir.InstDrain(name=f"ws-{n}", ins=[], outs=[])
                        n += 1
                        d.engine = inst.engine
                        d.sync_info = mybir.SyncInfo(on_wait=[w], on_update=[])
                        out.append(d)
                    si.on_wait = [waits[-1]]
                out.append(inst)
            if n:
                bb.instructions = out


def _build():
    import concourse.bass as bass
    import concourse.mybir as mybir
    from concourse import tile

    f32 = mybir.dt.float32
    bf16 = mybir.dt.bfloat16
    f8 = mybir.dt.float8e4
    Act = mybir.ActivationFunctionType

    nc = bass.Bass()
    xT = nc.dram_tensor("xT", [KC8, 128, 2, B], f8, kind="ExternalInput")
    mT = nc.dram_tensor("mT", [KC8, 128, 2, L], f8, kind="ExternalInput")
    zin_d = nc.dram_tensor("zin", [128, NBT * 4], f32, kind="ExternalOutput")
    topv_d = nc.dram_tensor("topv", [NBT, 128, NTOP], bf16, kind="ExternalOutput")

    EXP_SCALE = 1.0 / (FP8_SCALE * FP8_SCALE * T)

    with tile.TileContext(nc) as tc:
        with (
            tc.tile_pool(name="const", bufs=1) as cpool,
            tc.tile_pool(name="psum", bufs=8, space="PSUM") as ppool,
            tc.tile_pool(name="work", bufs=3) as wpool,
            tc.tile_pool(name="small", bufs=4) as spool,
        ):
            X = cpool.tile([128, KC8, 2, B], f8)
            M = cpool.tile([128, KC8, 2, L], f8)
            # All input DMAs ride ONE HWDGE ring (Sync), issued in exact
            # consumption order. HWDGE processes a ring FIFO, so each
            # chunk's completion sem fires at its byte-order position --
            # with inputs spread across two rings the SDMA engines
            # round-robin at packet granularity and every chunk completes
            # near the end of the whole transfer (2-3us sem waits at each
            # fill chunk boundary). M[0] goes in halves so the first
            # matmuls unblock sooner; M[7] in quarters so the last chunk's
            # per-bank matmuls stagger in as each quarter lands.
            nc.sync.dma_start(M[:, 0, :, 0:1024], mT[0, :, :, 0:1024])
            nc.sync.dma_start(X[:, 0, :, :], xT[0])
            nc.sync.dma_start(M[:, 0, :, 1024:2048], mT[0, :, :, 1024:2048])
            for kc in range(1, KC8 - 1):
                if kc < 3:
                    # early chunks in halves too: the PE catches up to the
                    # transfer here, so finer completion sems shave the
                    # per-boundary wait
                    nc.sync.dma_start(M[:, kc, :, 0:1024], mT[kc, :, :, 0:1024])
                    nc.sync.dma_start(
                        M[:, kc, :, 1024:2048], mT[kc, :, :, 1024:2048]
                    )
                else:
                    nc.sync.dma_start(M[:, kc, :, :], mT[kc])
                nc.sync.dma_start(X[:, kc, :, :], xT[kc])
            nc.sync.dma_start(X[:, KC8 - 1, :, :], xT[KC8 - 1])
            for q in range(4):
                sl = slice(q * 512, (q + 1) * 512)
                nc.sync.dma_start(M[:, KC8 - 1, :, sl], mT[KC8 - 1, :, :, sl])
            ZPALL = cpool.tile([128, NBT * 4], f32)

            # PE warm-up: HAM un-throttles after ~3.4us of sustained PE
            # activity. Throwaway matmuls on a zeroed scratch tile bridge
            # the gap until the first (X,M) chunk lands (~4us incl. HWDGE
            # latency), so real matmuls run at 2.4 GHz.
            GB = cpool.tile([128, 640], f8)
            nc.vector.memset(GB[:], 0.0)
            WARM = ppool.tile([128, 512], f32, tag="S")
            for _ in range(11):
                nc.tensor.matmul(
                    WARM[:], GB[:, 0:128], GB[:, 128:640],
                    start=True, stop=True,
                )

            # PSUM is managed as eight independent one-bank tiles (bufs=8):
            # Tile's pool-reuse dependency is buffer-granular, so per-bank
            # tiles let a btile's bank-k matmuls start as soon as the
            # bank-k exp of the btile two steps back has read its bank --
            # instead of waiting for the whole predecessor tile's 4-bank
            # exp/accumulator-read chain (a ~5us PE stall at fill end).
            # btiles 0 and 1 accumulate into all eight banks with their
            # matmuls interleaved in chunk-arrival order: each arriving
            # (X,M) chunk feeds 8 ready matmuls, keeping PE starvation at
            # the DMA-bw floor during the fill phase.
            S_a = [ppool.tile([128, 512], f32, tag="S", name=f"Sa{j}") for j in range(4)]
            S_b = [ppool.tile([128, 512], f32, tag="S", name=f"Sb{j}") for j in range(4)]
            S_pair = [S_a, S_b]
            for i in range(KC8):
                # order each chunk block to match sub-chunk arrival: the
                # tensor queue is in-order, so an MM needing a later
                # half/quarter must not sit ahead of ones whose data is
                # already resident
                if i == KC8 - 1:
                    # X7 precedes the quarters on the ring, so nch-major
                    # pairing leaves only 2 MMs queued behind the last
                    # quarter's sem instead of 5
                    order = [(bt, nch) for nch in range(4) for bt in range(2)]
                else:
                    # ring order within a chunk is M halves then X, so
                    # bt-major (b0's M-gated MMs before b1's X-gated ones)
                    # already matches arrival
                    order = [(bt, nch) for bt in range(2) for nch in range(4)]
                for bt, nch in order:
                    nc.tensor.matmul(
                        S_pair[bt][nch][:],
                        X[:, i, :, bt * 128 : (bt + 1) * 128],
                        M[:, i, :, nch * 512 : (nch + 1) * 512],
                        start=(i == 0),
                        stop=(i == KC8 - 1),
                        perf_mode=mybir.MatmulPerfMode.DoubleRow,
                    )

            def max8_chunk(E, cand, ch):
                nc.vector.max(
                    cand[:, ch * 8 : (ch + 1) * 8],
                    E[:, ch * CHW : (ch + 1) * CHW],
                )

            # Every btile's evacuation is per-512-bank: exp with the zin
            # part going to the ACT accumulator (the accumulator READ only
            # gates the tiny ZP write -- the PSUM bank itself is free the
            # moment the ACTIVATE ends, so the next-next btile's bank-k
            # start-matmul never waits on the read chain), then top-8 of
            # each 256 half on DVE. btiles >= 2 run their matmul groups
            # nch-major so bank k's evacuation fires while bank k+1's
            # matmuls stream; after the final matmul of the kernel only one
            # bank's exp+max8+reduce+DMA remains.
            for bt in range(NBT):
                E = wpool.tile([128, L], bf16, tag="E")
                cand = spool.tile([128, NCH * 8], bf16, tag="cand")
                if bt < 2:
                    S = S_pair[bt]
                else:
                    S = [ppool.tile([128, 512], f32, tag="S", name=f"S{bt}_{j}") for j in range(4)]
                for nch in range(4):
                    sl = slice(nch * 512, (nch + 1) * 512)
                    if bt >= 2:
                        for i in range(KC8):
                            nc.tensor.matmul(
                                S[nch][:],
                                X[:, i, :, bt * 128 : (bt + 1) * 128],
                                M[:, i, :, nch * 512 : (nch + 1) * 512],
                                start=(i == 0),
                                stop=(i == KC8 - 1),
                                perf_mode=mybir.MatmulPerfMode.DoubleRow,
                            )
                    # zin partials go straight into one persistent tile;
                    # the host sums the 4 columns per btile, so no device
                    # reduce sits between the last accumulator read and
                    # the zin DMA
                    nc.scalar.activation(
                        E[:, sl], S[nch][:], Act.Exp,
                        scale=EXP_SCALE,
                        accum_out=ZPALL[:, bt * 4 + nch : bt * 4 + nch + 1],
                    )
                    max8_chunk(E, cand, nch * 2)
                    max8_chunk(E, cand, nch * 2 + 1)
                    if bt == NBT - 1 and nch == 2:
                        # ship the first 3 banks' candidates early; the
                        # end-gating DMA is then only 4KB
                        nc.sync.dma_start(
                            topv_d[bt][:, 0:48], cand[:, 0:48]
                        )
                if bt < NBT - 1:
                    nc.sync.dma_start(topv_d[bt], cand[:])
                else:
                    # final outputs: last candidate bank on the (idle)
                    # Sync queue; zin issues from Scalar right after its
                    # last accumulator read -- same engine, no sem hop
                    nc.sync.dma_start(topv_d[bt][:, 48:64], cand[:, 48:64])
                    nc.scalar.dma_start(zin_d[:], ZPALL[:])

    _split_multi_waits(nc)
    return nc


def _get_nc():
    if "nc" not in _CACHE:
        _CACHE["nc"] = _build()
    return _CACHE["nc"]


def _pack_fp8(aT, ncols, f8):
    # [D, n] -> [KC8, 128, 2, n] with d = kc*256 + j*128 + p
    v = np.clip(aT * FP8_SCALE, -240.0, 240.0)
    v = v.reshape(KC8, 2, 128, ncols).transpose(0, 2, 1, 3)
    return np.ascontiguousarray(v).astype(f8)


def _prepare_in_maps(inputs, memory):
    import ml_dtypes

    f8 = ml_dtypes.float8_e4m3
    inputs = np.asarray(inputs, np.float32)
    memory = np.asarray(memory, np.float32)
    x = inputs / np.linalg.norm(inputs, axis=1, keepdims=True)
    xT = _pack_fp8(x.T, B, f8)
    in_maps = []
    for c in range(N_CAMS):
        mT = _pack_fp8(memory[c].T, L, f8)
        in_maps.append({"xT": xT, "mT": mT})
    return in_maps


def kernel(inputs, memory, indexes, cams_all, labels_all):
    from concourse.bass_utils import run_bass_kernel_spmd

    indexes = np.asarray(indexes).astype(np.int64)
    cams_all = np.asarray(cams_all).astype(np.int64)
    labels_all = np.asarray(labels_all).astype(np.int64)
    cams = cams_all[indexes]

    in_maps = _prepare_in_maps(inputs, memory)
    nc = _get_nc()
    res = run_bass_kernel_spmd(nc, in_maps, list(range(N_CAMS)))

    # epos = exp(S[t]/T) computed host-side from the same fp8-quantized
    # inputs the device consumed (f32 arithmetic ~= PSUM fp32 accumulate)
    tgts = labels_all[indexes]
    x8 = in_maps[0]["xT"].transpose(0, 2, 1, 3).reshape(D, B).astype(np.float32)
    epos = np.empty((N_CAMS, B), np.float64)
    for c in range(N_CAMS):
        m8 = in_maps[c]["mT"].transpose(0, 2, 1, 3).reshape(D, L).astype(np.float32)
        mt = m8[:, tgts]                     # [D, B]
        s_t = np.einsum("db,db->b", x8, mt, optimize=True)
        epos[c] = np.exp(s_t.astype(np.float64) / (FP8_SCALE * FP8_SCALE * T))

    # gather per-core stats; [128, NBT] -> [B] with b = bt*128 + p
    zin = np.empty((N_CAMS, B), np.float64)
    topv = np.empty((N_CAMS, B, NTOP), np.float64)
    for c in range(N_CAMS):
        r = res.results[c]
        zin[c] = (
            r["zin"].astype(np.float64).reshape(128, NBT, 4).sum(axis=2).T.reshape(B)
        )
        topv[c] = r["topv"].astype(np.float64).reshape(B, NTOP)

    # ---- intra: CE against own camera, mean within camera group, summed
    bidx = np.arange(B)
    bidx_all = bidx
    zin_own = zin[cams, bidx]
    epos_own = epos[cams, bidx]
    ce = np.log(zin_own) - np.log(epos_own)
    cnt = np.bincount(cams, minlength=N_CAMS).astype(np.float64)
    ce_sum = np.bincount(cams, weights=ce, minlength=N_CAMS)
    loss_intra = np.sum(ce_sum / np.maximum(cnt, 1.0))

    # remove the positive's own value from each camera's candidate list:
    # nearest candidate within 0.5% of the host-computed epos (device values
    # are bf16-rounded, so exact equality is not available)
    for c in range(N_CAMS):
        relerr = np.abs(topv[c] - epos[c][:, None]) / epos[c][:, None]
        j = np.argmin(relerr, axis=1)
        hit = relerr[bidx_all, j] < 5e-3
        topv[c][bidx_all[hit], j[hit]] = 0.0

    # ---- inter: exact global top-50 negatives from 8x56 candidates
    cand = topv[:, bidx, :].transpose(1, 0, 2).reshape(B, N_CAMS * NTOP)
    part = np.partition(cand, cand.shape[1] - HARD_NEG_K, axis=1)
    z50 = part[:, cand.shape[1] - HARD_NEG_K :].sum(axis=1)
    sum_epos = epos[:, bidx].sum(axis=0)
    lse = np.log(sum_epos + z50)
    mean_logpos = np.log(epos[:, bidx]).mean(axis=0)
    per_sample = lse - mean_logpos
    inter_sum = np.bincount(cams, weights=per_sample, minlength=N_CAMS)
    loss_inter = np.sum(inter_sum / np.maximum(cnt, 1.0)) * LOSS_WEIGHT

    return np.float32(loss_intra), np.float32(loss_inter)
